# revision 5
# baseline (speedup 1.0000x reference)
"""DualEncoder (two shared-weight LSTM encoders + bilinear score) on 8 trn2
NeuronCores.

Sharding: 8-way tensor parallelism over the 4H gate dimension; all 128
sequences (64 ctx + 64 resp) ride together as the matmul row dim. Per step
each core computes its 512-wide gate slice (bias + x_t proj + recurrent
proj accumulated in one PSUM bank), does the LSTM cell update on its
128-wide hidden slice, XBAR-DMA-transposes the new h slice into its hbuf
slot, and broadcasts it (bf16) to all peers. Arrival attribution uses
per-slot-group remote semaphores (slots 0-3 / 4-5 / 6-7) fed by sem-only
single-group broadcasts keyed off the sender's die, so the recurrent
matmuls start as slice groups land instead of waiting for the full
all-to-all. X embedding gathers are pipelined two steps ahead, cast to
bf16 on the scalar engine, and transposed by the Activation-queue XBAR DMA
(no PE transposes anywhere in the loop). sigmoid(diag(C @ M @ R^T)) is
computed replicated on every core at the end.
"""

import os

import numpy as np

N_CORES = 8
B = 64
T = 160
E = 512
H = 1024
V = 32000
GS = 512          # gate-slice width per core
S = 2 * B         # 128 sequences (ctx rows 0:64, resp rows 64:128)

LAST_EXEC_NS = None
_NC_CACHE = {}

# gate-chunk order inside the core's 512-wide slice: [i | f | o | g]
GATE_OFF = [0, H, 3 * H, 2 * H]


def _build(t_steps=T):
    from contextlib import ExitStack

    import concourse.bacc as bacc
    import concourse.bass as bass
    import concourse.mybir as mybir

    f32 = mybir.dt.float32
    bf16 = mybir.dt.bfloat16
    i32 = mybir.dt.int32

    nc = bacc.Bacc("TRN2", debug=False, num_devices=N_CORES,
                   monotonic_sem_count=4)

    d_ctx = nc.dram_tensor("contexts", [B, T], i32, kind="ExternalInput")
    d_rsp = nc.dram_tensor("responses", [B, T], i32, kind="ExternalInput")
    d_emb = nc.dram_tensor("emb", [V, E], f32, kind="ExternalInput")
    d_wih = nc.dram_tensor("Wih", [4 * H, E], f32, kind="ExternalInput")
    d_whh = nc.dram_tensor("Whh", [4 * H, H], f32, kind="ExternalInput")
    d_bih = nc.dram_tensor("bih", [1, 4 * H], f32, kind="ExternalInput")
    d_bhh = nc.dram_tensor("bhh", [1, 4 * H], f32, kind="ExternalInput")
    d_m = nc.dram_tensor("M", [H, H], f32, kind="ExternalInput")
    d_out = nc.dram_tensor("out", [1, B], f32, kind="ExternalOutput")

    arr = nc.monotonic_semaphore(0)   # data-broadcast remote sem (not waited)
    gL = nc.monotonic_semaphore(1)    # slots 0-3 ready (+8/step)
    gM = nc.monotonic_semaphore(2)    # slots 4,5 ready (+4/step)
    gH = nc.monotonic_semaphore(3)    # slots 6,7 ready (+4/step)

    es = ExitStack()
    sb = lambda name, shape, dt: es.enter_context(nc.sbuf_tensor(name, shape, dt))
    psa = lambda name, shape: es.enter_context(nc.psum_tensor(name, shape, f32))
    sem = lambda name: es.enter_context(nc.semaphore(name))

    tok = sb("tok", [S, T], i32)
    whhT = sb("whhT", [128, 8 * GS], bf16)
    wihT = sb("wihT", [128, 4 * GS], bf16)
    stagW = sb("stagW", [128, 4 * H], bf16)
    stagI = sb("stagI", [128, 4 * E], bf16)
    msb = sb("msb", [128, 8 * H], bf16)
    ones1 = sb("ones1", [1, 128], bf16)
    ones128 = sb("ones128", [128, 1], f32)
    bias = sb("bias", [1, GS], f32)
    btmp = sb("btmp", [1, GS], f32)
    bias16 = sb("bias16", [1, GS], bf16)
    xraw = sb("xraw", [128, 2 * E], f32)
    x16 = sb("x16", [128, 2 * E], bf16)
    xt_sb = sb("xt_sb", [128, 2 * E], bf16)
    hbuf = sb("hbuf", [128, 2 * H], bf16)
    h16 = sb("h16", [128, 2 * 128], bf16)
    sig_sb = sb("sig_sb", [128, 2 * 384], f32)
    tg_sb = sb("tg_sb", [128, 2 * 128], f32)
    tc_sb = sb("tc_sb", [128, 2 * 128], f32)
    c_sb = sb("c_sb", [128, 128], f32)
    t1_sb = sb("t1_sb", [128, 128], f32)
    t2_sb = sb("t2_sb", [128, 128], f32)
    rh_sb = sb("rh_sb", [128, 8 * B], f32)
    zw_sb = sb("zw_sb", [128, 8 * B], f32)
    out_sb = sb("out_sb", [1, B], f32)

    gates_ps = psa("gates_ps", [128, 2 * GS])
    z_ps = psa("z_ps", [128, 8 * B])
    s_ps = psa("s_ps", [1, B])

    s_sync = sem("s_sync")
    s_gset = sem("s_gset")
    s_wdma = sem("s_wdma")
    s_wt = sem("s_wt")
    s_gdma = sem("s_gdma")
    s_gather = sem("s_gather")
    s_bias = sem("s_bias")
    s_xc = sem("s_xc")
    s_xt = sem("s_xt")
    s_xpd = sem("s_xpd")
    s_gates = sem("s_gates")
    s_acts = sem("s_acts")
    s_c = sem("s_c")
    s_tc = sem("s_tc")
    s_h = sem("s_h")
    s_htp = sem("s_htp")
    s_prep = sem("s_prep")
    s_send = sem("s_send")
    s_z = sem("s_z")
    s_zmul = sem("s_zmul")
    s_zred = sem("s_zred")
    s_out = sem("s_out")
    s_fin = sem("s_fin")

    p_last = (t_steps - 1) % 2

    def xt_view(bank):
        return xt_sb[:, E * bank : E * (bank + 1)].rearrange(
            "a (b c) -> a b c", b=4
        )

    with nc.Block() as block:

        # ---------------- SYNC (HWDGE #1): setup, weight transposes,
        # per-step h transposes, final store -------------------------------
        @block.sync
        def _(sync):
            pid = sync.partition_id()
            sync.dma_start(tok[0:B, :], d_ctx[:, :]).then_inc(s_sync, 16)
            sync.dma_start(tok[B:S, :], d_rsp[:, :]).then_inc(s_sync, 16)
            for m in range(4):  # -> 96
                sync.dma_start(
                    bias[:, 128 * m : 128 * (m + 1)],
                    d_bih[:, bass.ds(pid * 128 + GATE_OFF[m], 128)],
                ).then_inc(s_sync, 16)
            for m in range(4):  # -> 160
                sync.dma_start(
                    btmp[:, 128 * m : 128 * (m + 1)],
                    d_bhh[:, bass.ds(pid * 128 + GATE_OFF[m], 128)],
                ).then_inc(s_sync, 16)
            # weight transposes (cast loads done by gpsimd)
            sync.wait_ge(s_wdma, 64)
            for g in range(4):
                for j in range(8):
                    sync.dma_start_transpose(
                        whhT[:, GS * j + 128 * g : GS * j + 128 * (g + 1)],
                        stagW[:, H * g + 128 * j : H * g + 128 * (j + 1)],
                    ).then_inc(s_wt, 16)  # -> 512
            sync.wait_ge(s_wdma, 128)
            for g in range(4):
                for e in range(4):
                    sync.dma_start_transpose(
                        wihT[:, GS * e + 128 * g : GS * e + 128 * (g + 1)],
                        stagI[:, E * g + 128 * e : E * g + 128 * (e + 1)],
                    ).then_inc(s_wt, 16)  # -> 768
            # per-step h transposes into the local hbuf slot
            for t in range(t_steps):
                p = t % 2
                sync.wait_ge(s_h, t + 1)
                sync.dma_start_transpose(
                    hbuf[:, bass.ds(H * p + pid * 128, 128)],
                    h16[:, 128 * p : 128 * (p + 1)],
                ).then_inc(s_htp, 16)
            sync.wait_ge(s_out, 1)
            sync.dma_start(d_out[:, :], out_sb[:, :]).then_inc(s_fin, 16)
            sync.wait_ge(s_fin, 16)

        # ---------------- GPSIMD: setup loads, gathers, broadcast preps ----
        @block.gpsimd
        def _(gpsimd):
            pid = gpsimd.partition_id()
            gpsimd.memset(ones1[:, :], 1.0)
            gpsimd.memset(ones128[:, :], 1.0)
            gpsimd.memset(c_sb[:, :], 0.0)
            gpsimd.sem_inc(s_gset, 1)
            for g in range(4):  # Whh cast loads, f32 -> bf16 via SWDGE
                gpsimd.dma_start(
                    stagW[:, H * g : H * (g + 1)],
                    d_whh[bass.ds(pid * 128 + GATE_OFF[g], 128), :],
                ).then_inc(s_wdma, 16)
            for g in range(4):  # Wih cast loads
                gpsimd.dma_start(
                    stagI[:, E * g : E * (g + 1)],
                    d_wih[bass.ds(pid * 128 + GATE_OFF[g], 128), :],
                ).then_inc(s_wdma, 16)
            for i in range(8):  # M cast load
                gpsimd.dma_start(
                    msb[:, H * i : H * (i + 1)], d_m[128 * i : 128 * (i + 1), :]
                ).then_inc(s_gdma, 16)

            gpsimd.wait_ge(s_sync, 32)
            for t0 in range(min(2, t_steps)):  # prologue gathers
                gpsimd.indirect_dma_start(
                    out=xraw[:, E * t0 : E * (t0 + 1)],
                    out_offset=None,
                    in_=d_emb[:, :],
                    in_offset=bass.IndirectOffsetOnAxis(
                        ap=tok[:, t0 : t0 + 1], axis=0
                    ),
                ).then_inc(s_gather, 16)

            rdests = [None] + [(0, k) for k in range(1, N_CORES)]
            rd_all = [(0, k) for k in range(N_CORES)]
            for t in range(t_steps):
                p = t % 2
                if t + 2 < t_steps:
                    gpsimd.wait_ge(s_xc, t + 1)       # xraw bank p free
                    if t >= 1:
                        gpsimd.wait_ge(s_send, 32 * t)  # prior sends drained
                    gpsimd.indirect_dma_start(
                        out=xraw[:, E * p : E * (p + 1)],
                        out_offset=None,
                        in_=d_emb[:, :],
                        in_offset=bass.IndirectOffsetOnAxis(
                            ap=tok[:, t + 2 : t + 3], axis=0
                        ),
                    ).then_inc(s_gather, 16)
                own = hbuf[:, bass.ds(H * p + pid * 128, 128)]
                gpsimd.remote_dma_broadcast(
                    out_ap=own,
                    in_ap=own,
                    remote_sem=arr.sem(),
                    local_sem=s_send,
                    rdests=rdests,
                ).then_inc(s_prep, 1)
                with gpsimd.If(pid < 4):
                    gpsimd.remote_sem_update_broadcast(
                        remote_sem=gL.sem(), local_sem=s_send, rdests=rd_all
                    ).then_inc(s_prep, 1)
                with gpsimd.Else():
                    with gpsimd.If(pid < 6):
                        gpsimd.remote_sem_update_broadcast(
                            remote_sem=gM.sem(), local_sem=s_send, rdests=rd_all
                        ).then_inc(s_prep, 1)
                    with gpsimd.Else():
                        gpsimd.remote_sem_update_broadcast(
                            remote_sem=gH.sem(), local_sem=s_send, rdests=rd_all
                        ).then_inc(s_prep, 1)
                gpsimd.wait_ge(s_prep, 2 * (t + 1))
                gpsimd.wait_ge(s_htp, 16 * (t + 1))
                gpsimd.trigger_dma(count=2)

        # ---------------- PE: xproj + staged recurrent matmuls -------------
        @block.tensor
        def _(pe):
            pe_pid = pe.partition_id()
            pe.wait_ge(s_wt, 768)
            pe.wait_ge(s_gset, 1)
            pe.wait_ge(s_bias, 1)
            pe.wait_ge(s_xt, 16)
            # prologue: gates(0) = bias + xproj(0) into bank 0
            nc.tensor.matmul(
                gates_ps[:, 0:GS], ones1[:, :], bias16[:, :],
                start=True, stop=False, skip_group_check=True,
            )
            for e in range(4):
                mm = nc.tensor.matmul(
                    gates_ps[:, 0:GS],
                    xt_sb[:, 128 * e : 128 * (e + 1)],
                    wihT[:, GS * e : GS * (e + 1)],
                    start=False,
                    stop=(e == 3),
                    skip_group_check=True,
                )
                if e == 3:
                    mm.then_inc(s_gates, 1)
                elif e == 2:
                    mm.then_inc(s_xpd, 1)

            def stage(slots, t, p, pm, close):
                for n, j in enumerate(slots):
                    mm = nc.tensor.matmul(
                        gates_ps[:, GS * p : GS * (p + 1)],
                        hbuf[:, H * pm + 128 * j : H * pm + 128 * (j + 1)],
                        whhT[:, GS * j : GS * (j + 1)],
                        start=False,
                        stop=(close and n == len(slots) - 1),
                        skip_group_check=True,
                    )
                    if close and n == len(slots) - 1:
                        mm.then_inc(s_gates, 1)

            for t in range(t_steps):
                p = t % 2
                q = (t + 1) % 2
                if t + 1 <= t_steps - 1:
                    # bias + xproj for gates(t+1) into bank q (early, fills
                    # the arrival-wait window)
                    pe.wait_ge(s_acts, 3 * t)       # bank q drained by ACT
                    pe.wait_ge(s_xt, 16 * (t + 2))  # xt bank q ready
                    nc.tensor.matmul(
                        gates_ps[:, GS * q : GS * (q + 1)],
                        ones1[:, :],
                        bias16[:, :],
                        start=True,
                        stop=False,
                        skip_group_check=True,
                    )
                    for e in range(4):
                        mm = nc.tensor.matmul(
                            gates_ps[:, GS * q : GS * (q + 1)],
                            xt_sb[:, E * q + 128 * e : E * q + 128 * (e + 1)],
                            wihT[:, GS * e : GS * (e + 1)],
                            start=False,
                            stop=False,
                            skip_group_check=True,
                        )
                        if e == 3:
                            mm.then_inc(s_xpd, 1)
                if t >= 1:
                    pm = (t - 1) % 2
                    with pe.If(pe_pid < 4):
                        pe.wait_ge(gL.sem(), 8 * t)
                        stage([0, 1, 2, 3], t, p, pm, False)
                        pe.wait_ge(gM.sem(), 4 * t)
                        stage([4, 5], t, p, pm, False)
                        pe.wait_ge(gH.sem(), 4 * t)
                        stage([6, 7], t, p, pm, True)
                    with pe.Else():
                        pe.wait_ge(gM.sem(), 4 * t)
                        stage([4, 5], t, p, pm, False)
                        pe.wait_ge(gH.sem(), 4 * t)
                        stage([6, 7], t, p, pm, False)
                        pe.wait_ge(gL.sem(), 8 * t)
                        stage([0, 1, 2, 3], t, p, pm, True)

            # ---------------- bilinear epilogue ----------------
            pe.wait_ge(gL.sem(), 8 * t_steps)
            pe.wait_ge(gM.sem(), 4 * t_steps)
            pe.wait_ge(gH.sem(), 4 * t_steps)
            pe.wait_ge(s_gdma, 128)
            for jm in range(8):
                for im in range(8):
                    mm = nc.tensor.matmul(
                        z_ps[:, B * jm : B * (jm + 1)],
                        msb[:, H * im + 128 * jm : H * im + 128 * (jm + 1)],
                        hbuf[:, H * p_last + 128 * im : H * p_last + 128 * im + B],
                        start=(im == 0),
                        stop=(im == 7),
                    )
                    if jm == 7 and im == 7:
                        mm.then_inc(s_z, 1)
            pe.wait_ge(s_zmul, 1)
            for jm in range(8):
                mm = nc.tensor.matmul(
                    s_ps[:, :],
                    ones128[:, :],
                    zw_sb[:, B * jm : B * (jm + 1)],
                    start=(jm == 0),
                    stop=(jm == 7),
                )
                if jm == 7:
                    mm.then_inc(s_zred, 1)

        # ---------------- ACT (scalar, HWDGE #2): activations, x casts,
        # x transposes ------------------------------------------------------
        @block.scalar
        def _(act):
            import concourse.mybir as mybir

            AF = mybir.ActivationFunctionType
            # prologue: cast + transpose x(0), x(1)
            for t0 in range(min(2, t_steps)):
                act.wait_ge(s_gather, 16 * (t0 + 1))
                nc.scalar.activation(
                    x16[:, E * t0 : E * (t0 + 1)],
                    xraw[:, E * t0 : E * (t0 + 1)],
                    AF.Copy,
                ).then_inc(s_xc, 1)
                act.dma_start_transpose(
                    xt_view(t0), x16[:, E * t0 : E * (t0 + 1)]
                ).then_inc(s_xt, 16)

            for t in range(t_steps):
                p = t % 2
                act.wait_ge(s_gates, t + 1)
                nc.scalar.activation(
                    sig_sb[:, 384 * p : 384 * p + 256],  # sigmoid(i), sigmoid(f)
                    gates_ps[:, GS * p : GS * p + 256],
                    AF.Sigmoid,
                ).then_inc(s_acts, 1)
                nc.scalar.activation(
                    tg_sb[:, 128 * p : 128 * (p + 1)],  # tanh(g)
                    gates_ps[:, GS * p + 384 : GS * (p + 1)],
                    AF.Tanh,
                ).then_inc(s_acts, 1)
                nc.scalar.activation(
                    sig_sb[:, 384 * p + 256 : 384 * p + 384],  # sigmoid(o)
                    gates_ps[:, GS * p + 256 : GS * p + 384],
                    AF.Sigmoid,
                ).then_inc(s_acts, 1)
                act.wait_ge(s_c, t + 1)
                nc.scalar.activation(
                    tc_sb[:, 128 * p : 128 * (p + 1)],
                    c_sb[:, :],
                    AF.Tanh,
                ).then_inc(s_tc, 1)
                if t + 2 <= t_steps - 1:
                    act.wait_ge(s_xt, 16 * (t + 2))      # x16 bank p free
                    act.wait_ge(s_gather, 16 * (t + 3))  # gather(t+2) landed
                    nc.scalar.activation(
                        x16[:, E * p : E * (p + 1)],
                        xraw[:, E * p : E * (p + 1)],
                        AF.Copy,
                    ).then_inc(s_xc, 1)
                    act.wait_ge(s_xpd, t + 1)            # xproj(t) consumed
                    act.dma_start_transpose(
                        xt_view(p), x16[:, E * p : E * (p + 1)]
                    ).then_inc(s_xt, 16)

            # epilogue sigmoid
            act.wait_ge(s_zred, 1)
            nc.scalar.activation(out_sb[:, :], s_ps[:, :], AF.Sigmoid).then_inc(
                s_out, 1
            )

        # ---------------- DVE (vector): cell update ------------------------
        @block.vector
        def _(dve):
            # bias add
            dve.wait_ge(s_sync, 160)
            nc.vector.tensor_add(bias[:, :], bias[:, :], btmp[:, :])
            nc.vector.tensor_copy(bias16[:, :], bias[:, :]).then_inc(s_bias, 1)
            dve.wait_ge(s_gset, 1)

            for t in range(t_steps):
                p = t % 2
                if t >= 2:
                    dve.wait_ge(s_htp, 16 * (t - 1))  # h16 bank p free
                dve.wait_ge(s_acts, 3 * t + 1)
                nc.vector.tensor_mul(
                    t1_sb[:, :],
                    sig_sb[:, 384 * p + 128 : 384 * p + 256],  # sigmoid(f)
                    c_sb[:, :],
                )
                dve.wait_ge(s_acts, 3 * t + 2)
                nc.vector.tensor_mul(
                    t2_sb[:, :],
                    sig_sb[:, 384 * p : 384 * p + 128],  # sigmoid(i)
                    tg_sb[:, 128 * p : 128 * (p + 1)],
                )
                nc.vector.tensor_add(c_sb[:, :], t1_sb[:, :], t2_sb[:, :]).then_inc(
                    s_c, 1
                )
                dve.wait_ge(s_acts, 3 * t + 3)
                dve.wait_ge(s_tc, t + 1)
                nc.vector.tensor_mul(
                    h16[:, 128 * p : 128 * (p + 1)],
                    sig_sb[:, 384 * p + 256 : 384 * p + 384],  # sigmoid(o)
                    tc_sb[:, 128 * p : 128 * (p + 1)],
                ).then_inc(s_h, 1)

            # epilogue: rh copy + elementwise mul
            dve.wait_ge(s_z, 1)
            for jm in range(8):
                nc.vector.tensor_copy(
                    rh_sb[:, B * jm : B * (jm + 1)],
                    hbuf[:, H * p_last + 128 * jm + B : H * p_last + 128 * (jm + 1)],
                )
            for jm in range(8):
                ins = nc.vector.tensor_mul(
                    zw_sb[:, B * jm : B * (jm + 1)],
                    z_ps[:, B * jm : B * (jm + 1)],
                    rh_sb[:, B * jm : B * (jm + 1)],
                )
                if jm == 7:
                    ins.then_inc(s_zmul, 1)

    es.close()
    nc.compile()
    return nc


def _get_nc(t_steps=T):
    if t_steps not in _NC_CACHE:
        _NC_CACHE[t_steps] = _build(t_steps)
    return _NC_CACHE[t_steps]


def kernel(**inputs):
    global LAST_EXEC_NS
    from concourse.bass_utils import run_bass_kernel_spmd

    t_steps = int(os.environ.get("BASS_KERNEL_TSTEPS", str(T)))
    nc = _get_nc(t_steps)
    in_map = {
        "contexts": np.ascontiguousarray(np.asarray(inputs["contexts"], np.int32)),
        "responses": np.ascontiguousarray(np.asarray(inputs["responses"], np.int32)),
        "emb": np.ascontiguousarray(np.asarray(inputs["emb"], np.float32)),
        "Wih": np.ascontiguousarray(np.asarray(inputs["Wih"], np.float32)),
        "Whh": np.ascontiguousarray(np.asarray(inputs["Whh"], np.float32)),
        "bih": np.ascontiguousarray(
            np.asarray(inputs["bih"], np.float32).reshape(1, 4 * H)
        ),
        "bhh": np.ascontiguousarray(
            np.asarray(inputs["bhh"], np.float32).reshape(1, 4 * H)
        ),
        "M": np.ascontiguousarray(np.asarray(inputs["M"], np.float32)),
    }
    res = run_bass_kernel_spmd(
        nc,
        [dict(in_map) for _ in range(N_CORES)],
        core_ids=list(range(N_CORES)),
        trace=bool(int(os.environ.get("BASS_KERNEL_TRACE", "0"))),
        trace_cores=(
            list(range(N_CORES))
            if int(os.environ.get("BASS_KERNEL_TRACE_ALL", "0"))
            else None
        ),
    )
    LAST_EXEC_NS = res.exec_time_ns
    return res.results[0]["out"].reshape(B).astype(np.float32)
